# revision 36
# baseline (speedup 1.0000x reference)
"""CausalSelfAttentionWithMemory on 8 TRN2 NeuronCores — v3.

Sharding: core = 2*b + hg  (b in 0..3 batches, hg in 0..1 head-groups of 8
heads).  Each core computes qkv for its batch/head-group, attention, and the
partial c_proj (its 512 rows of W_proj); partials are pair-reduced with
chunked bf16 ReduceScatters; rs -> out copies are scheduled two stretches
later so no DMA queue ever waits behind an in-flight collective.

v3 changes vs v2 (374us -> 344us):
  - chunk order c0..c4 by ascending key range: attention (and the scalar
    engine's exp stream) starts right after a minimal upfront projection
    (tokens 0:640 only, wq loaded in two m-group halves); the rest of
    phase 1 drains as PE filler inside the early attention stretches,
    c_proj of chunk i drains inside chunk i+1.
  - last 512 queries split 384 (c3) + 128 (c4) so c3's ReduceScatter hides
    under c4's attention; only c4's small RS (136 rows) is exposed as tail,
    whose c_proj cycles PSUM through the freed attention slots.
  - the 8 memory queries are folded into c4 as extra score columns (same
    key range: everything) — removes v2's 272 tiny memq matmuls.
  - c4's exp is batched 3 key-chunks per ACTIVATE; sub-slots are packed
    inside the standard [128,2,512] PSUM tile because a matmul output must
    never straddle a 2KB PSUM bank boundary (hardware fault otherwise).
  - reciprocal_approx_fast/partition_broadcast run at the proven 512 width
    (width 136 hard-faulted the device; lanes beyond C4W are never read).

Known non-fixables found in profiling: the PE is power-throttled to 50%
for ~16% of the run (throttle_activity_1), and ACTIVATE has a ~250ns fixed
cost per instruction; PE busy ~276us of the ~344us wall.

Layouts on device (per core):
  xT    (1024, 2056) bf16  x[b] transposed (contraction dim on partitions)
  w_qk  (1024, 1024) bf16  [q-cols | k-cols] of this head-group
  w_v   (1024, 512)  bf16  v-cols
  w_p   (512, 1024)  bf16  W_proj rows of this head-group
  masks (5, 128, 1024) bf16 boundary masks: mask[j][r, hl*512+c] = r <= c+8-128j
  maskc4 (2, 128, 272) bf16 c4 masks over cols [mem 0:8 | seq 1928:2056]

Attention in S^T (key, query) layout: S^T tile = matmul(lhsT=k^T, rhs=q^T);
exp without max-subtraction (scores bounded); multiplicative causal mask;
AV via lhsT=[v|1] augmented (ones column = softmax denominator, row 64).
"""

import numpy as np

B, L, C = 4, 2056, 1024
HD = 64
KMEM = 8
HPC = 8  # heads per core
NKC = 17  # key chunks of 128 (last has 8 rows)

_cache = {}

# chunks: (q0, W, nfull) for the four seq chunks; c4 is special.
CHUNKS = [(8, 512, 0), (520, 512, 4), (1032, 512, 8), (1544, 384, 12)]
# c_proj/RS segments: seg[c] = (pieces, R, out_off); pieces = [(row0, nrows)].
SEGS = [
    ([(8, 512)], 512, 0),
    ([(520, 512)], 512, 256),
    ([(1032, 512)], 512, 512),
    ([(1544, 384)], 384, 768),
    ([(0, 8), (1928, 128)], 136, 960),
]
OUT_ROWS = 1028
C4W = 136  # 8 mem + 128 seq query columns


def _emit(tc, xT, w_qk, w_v, w_p, masks_d, maskc4_d, out_ext):
    import concourse.bass as bass  # noqa: F401
    from concourse import mybir

    nc = tc.nc
    f32 = mybir.dt.float32
    bf16 = mybir.dt.bfloat16
    EXP = mybir.ActivationFunctionType.Exp

    with (
        tc.tile_pool(name="res", bufs=1) as res_pool,
        tc.tile_pool(name="dram", bufs=1, space="DRAM") as d_pool,
    ):
        # ---- resident tensors; DMAs ordered to unblock the upfront phase ----
        wv = [res_pool.tile([128, 512], bf16, name=f"wv{cc}") for cc in range(8)]
        for cc in range(8):
            nc.sync.dma_start(wv[cc], w_v[128 * cc : 128 * cc + 128, :])
        x_sb = res_pool.tile([128, 8, L], bf16, name="x_sb")
        for cc in range(8):  # tokens 0:128 first so 1a(l0) starts earliest
            nc.sync.dma_start(x_sb[:, cc, 0:128], xT[128 * cc : 128 * cc + 128, 0:128])
        for cc in range(8):
            nc.sync.dma_start(
                x_sb[:, cc, 128:512], xT[128 * cc : 128 * cc + 128, 128:512]
            )
        # wq split in two half-loads: pairs 0/1 columns (m 0,4,1,5) first so
        # the first 1b matmuls start after 1MB instead of 2MB.
        wq = [res_pool.tile([128, 1024], bf16, name=f"wq{cc}") for cc in range(8)]
        for cc in range(8):
            nc.sync.dma_start(
                wq[cc].rearrange("p (h c) -> p h c", h=2)[:, :, 0:256],
                w_qk[128 * cc : 128 * cc + 128, :]
                .rearrange("p (h c) -> p h c", h=2)[:, :, 0:256],
            )
        mk = [res_pool.tile([128, 2, 512], bf16, name=f"mk{j}") for j in range(5)]
        for cc in range(8):
            nc.sync.dma_start(
                x_sb[:, cc, 512:640], xT[128 * cc : 128 * cc + 128, 512:640]
            )
        for cc in range(8):
            nc.sync.dma_start(
                wq[cc].rearrange("p (h c) -> p h c", h=2)[:, :, 256:512],
                w_qk[128 * cc : 128 * cc + 128, :]
                .rearrange("p (h c) -> p h c", h=2)[:, :, 256:512],
            )
        for j in range(5):
            nc.sync.dma_start(mk[j].rearrange("p a b -> p (a b)"), masks_d[j])
        mk4 = [res_pool.tile([128, 2, C4W], bf16, name=f"mk4{j}") for j in range(2)]
        for j in range(2):
            nc.sync.dma_start(mk4[j].rearrange("p a b -> p (a b)"), maskc4_d[j])
        wp = [res_pool.tile([128, 1024], bf16, name=f"wp{rr}") for rr in range(4)]
        for rr in range(4):
            nc.sync.dma_start(wp[rr], w_p[128 * rr : 128 * rr + 128, :])
        for n0, nw in [(640, 512), (1152, 512), (1664, 384), (2048, 8)]:
            for cc in range(8):
                nc.sync.dma_start(
                    x_sb[:, cc, n0 : n0 + nw],
                    xT[128 * cc : 128 * cc + 128, n0 : n0 + nw],
                )

        qkT = [res_pool.tile([128, L], bf16, name=f"qkT{m}") for m in range(8)]
        vt = [res_pool.tile([128, HPC, HD + 1], bf16, name=f"vt{l}") for l in range(NKC)]
        yt = [res_pool.tile([128, L], bf16, name=f"yt{p}") for p in range(4)]
        qm = [res_pool.tile([128, C4W], bf16, name=f"qm{p}") for p in range(4)]

        MORD = [0, 4, 1, 5, 2, 6, 3, 7]  # q/k tiles of pair p land together

        # ---- upfront: only what chunk c0 needs (q/k tokens 0:640, v 0:640) ----
        with tc.tile_pool(name="ps1", bufs=4, space="PSUM") as ps1:
            # PE warmup during the initial DMA wait lifts HAM to K=8/8.
            warm = res_pool.tile([128, 512], bf16, name="warm")
            nc.vector.memset(warm, 0.0)
            pw = ps1.tile([128, 512], f32, tag="ps", name="pw")
            for i in range(16):
                nc.tensor.matmul(
                    pw, warm[:, 0:128], warm, start=(i == 0), stop=(i == 15)
                )

            def p1a(l):
                lw = min(128, L - 128 * l)
                ps = ps1.tile([128, 512], f32, tag="ps", name="pst")
                for cc in range(8):
                    nc.tensor.matmul(
                        ps[:lw, :],
                        x_sb[:, cc, 128 * l : 128 * l + lw],
                        wv[cc],
                        start=(cc == 0),
                        stop=(cc == 7),
                    )
                nc.vector.tensor_copy(
                    vt[l][:lw, :, 0:HD],
                    ps[:lw, :].rearrange("p (h d) -> p h d", h=HPC),
                )
                nc.vector.memset(vt[l][:, :, HD : HD + 1], 1.0)

            def p1b(n0, nw, m):
                ps = ps1.tile([128, 512], f32, tag="ps", name="pst")
                for cc in range(8):
                    nc.tensor.matmul(
                        ps[:, :nw],
                        wq[cc][:, 128 * m : 128 * m + 128],
                        x_sb[:, cc, n0 : n0 + nw],
                        start=(cc == 0),
                        stop=(cc == 7),
                    )
                nc.vector.tensor_copy(qkT[m][:, n0 : n0 + nw], ps[:, :nw])

            for l in range(5):
                p1a(l)
            for m in MORD:
                p1b(0, 512, m)
            for m in MORD:
                p1b(512, 128, m)

        # ---- attention chunks + chunked c_proj/ReduceScatter ----
        with (
            tc.tile_pool(name="psS", bufs=2, space="PSUM") as psS,
            tc.tile_pool(name="psAV", bufs=3, space="PSUM") as psAV,
            tc.tile_pool(name="ps3", bufs=1, space="PSUM") as ps3,
            tc.tile_pool(name="sexp", bufs=4) as se_pool,
            tc.tile_pool(name="small", bufs=2) as sm_pool,
            tc.tile_pool(name="ostage", bufs=4) as o_pool,
        ):
            # -- phase-1 remainder as single-instruction filler closures --
            def fillers_1a(l):
                lw = min(128, L - 128 * l)
                ps = ps3.tile([128, 512], f32, tag="p3", name="pft")

                def mm(cc):
                    def run():
                        nc.tensor.matmul(
                            ps[:lw, :],
                            x_sb[:, cc, 128 * l : 128 * l + lw],
                            wv[cc],
                            start=(cc == 0),
                            stop=(cc == 7),
                        )

                    return run

                def fin():
                    nc.vector.tensor_copy(
                        vt[l][:lw, :, 0:HD],
                        ps[:lw, :].rearrange("p (h d) -> p h d", h=HPC),
                    )
                    nc.vector.memset(vt[l][:, :, HD : HD + 1], 1.0)

                return [mm(cc) for cc in range(8)] + [fin]

            def fillers_1b(n0, nw, m):
                ps = ps3.tile([128, 512], f32, tag="p3", name="pft")

                def mm(cc):
                    def run():
                        nc.tensor.matmul(
                            ps[:, :nw],
                            wq[cc][:, 128 * m : 128 * m + 128],
                            x_sb[:, cc, n0 : n0 + nw],
                            start=(cc == 0),
                            stop=(cc == 7),
                        )

                    return run

                def fin():
                    nc.vector.tensor_copy(qkT[m][:, n0 : n0 + nw], ps[:, :nw])

                return [mm(cc) for cc in range(8)] + [fin]

            def normalize(avs, p, q0, qw):
                """yt[p][:, q0:q0+qw] = avs rows/den (row 64 = denominator)."""
                for hl in range(2):
                    den = sm_pool.tile([1, 512], f32, tag="den", name="dent")
                    nc.vector.tensor_copy(den[:, :qw], avs[hl][HD : HD + 1, :qw])
                    inv = sm_pool.tile([1, 512], f32, tag="inv", name="invt")
                    nc.vector.reciprocal_approx_fast(inv[:, :qw], den[:, :qw])
                    bc = sm_pool.tile([64, 512], f32, tag="bc", name="bct")
                    nc.gpsimd.partition_broadcast(bc[:, :qw], inv[:, :qw])
                    nc.vector.tensor_mul(
                        yt[p][64 * hl : 64 * hl + 64, q0 : q0 + qw],
                        avs[hl][0:HD, :qw],
                        bc[:, :qw],
                    )

            def normalize_c4(avs, p):
                """c4 columns map to yt cols [0:8 | 1928:2056].  den/recip/
                broadcast run at the v2-proven 512 width (lanes beyond C4W
                hold garbage and are never read)."""
                for hl in range(2):
                    den = sm_pool.tile([1, 512], f32, tag="den", name="dent")
                    nc.vector.tensor_copy(den, avs[hl][HD : HD + 1, :])
                    inv = sm_pool.tile([1, 512], f32, tag="inv", name="invt")
                    nc.vector.reciprocal_approx_fast(inv, den)
                    bc = sm_pool.tile([64, 512], f32, tag="bc", name="bct")
                    nc.gpsimd.partition_broadcast(bc, inv)
                    row = 64 * hl
                    nc.vector.tensor_mul(
                        yt[p][row : row + 64, 0:KMEM],
                        avs[hl][0:HD, 0:KMEM],
                        bc[:, 0:KMEM],
                    )
                    nc.vector.tensor_mul(
                        yt[p][row : row + 64, 1928:2056],
                        avs[hl][0:HD, KMEM:C4W],
                        bc[:, KMEM:C4W],
                    )

            def phase3_seg(pieces, R, out_off, key):
                """c_proj for the given row pieces as single-matmul closures,
                then a ReduceScatter into an internal DRAM tile.  The rs ->
                out_ext copy is scheduled later (copies list) so no DMA queue
                ever waits behind an in-flight collective."""
                partial = d_pool.tile([R, C], bf16, name=f"partial{key}")
                units = []
                tlist = []  # (src_row0, dst_row0, lw)
                dst = 0
                for row0, nrows in pieces:
                    for t0 in range(0, nrows, 128):
                        lw = min(128, nrows - t0)
                        tlist.append((row0 + t0, dst, lw))
                        dst += lw
                psum_cycle = [(ps3, "p3")]
                if key == "c4":  # attention pools are free during the tail
                    psum_cycle = [
                        (ps3, "p3"),
                        (psS, "st"),
                        (psS, "st"),
                        (psAV, "av"),
                        (psAV, "av"),
                    ]
                ci = 0
                for src0, dst0, lw in tlist:
                    for n in range(2):
                        pool, ptag = psum_cycle[ci % len(psum_cycle)]
                        ci += 1
                        ps = pool.tile([128, 512], f32, tag=ptag, name="ps3t")

                        def mm(rr, ps=ps, src0=src0, n=n, lw=lw):
                            def run():
                                nc.tensor.matmul(
                                    ps[:lw, :],
                                    yt[rr][:, src0 : src0 + lw],
                                    wp[rr][:, 512 * n : 512 * n + 512],
                                    start=(rr == 0),
                                    stop=(rr == 3),
                                )

                            return run

                        def fin(ps=ps, dst0=dst0, n=n, lw=lw):
                            ost = o_pool.tile([128, 512], bf16, tag="ost", name="ostt")
                            nc.vector.tensor_copy(ost[:lw, :], ps[:lw, :])
                            nc.sync.dma_start(
                                partial[dst0 : dst0 + lw, 512 * n : 512 * n + 512],
                                ost[:lw, :],
                            )

                        units += [mm(rr) for rr in range(4)]
                        units.append(fin)

                rs = d_pool.tile([R // 2, C], bf16, name=f"rs{key}")

                def finish():
                    nc.gpsimd.collective_compute(
                        "ReduceScatter",
                        mybir.AluOpType.add,
                        replica_groups=[[0, 1], [2, 3], [4, 5], [6, 7]],
                        ins=[partial.opt()],
                        outs=[rs.opt()],
                    )

                def copy_out():
                    nc.sync.dma_start(out_ext[out_off : out_off + R // 2, :], rs)

                units.append(finish)
                return units, copy_out

            pending = []  # filler closures drained into the attention stream

            def drain(n):
                for _ in range(min(n, len(pending))):
                    pending.pop(0)()

            def drain_paced(iters_left):
                """Spread the remaining fillers evenly over the remaining
                exp/AV iterations of the stretch so the PE never runs dry."""
                if iters_left <= 1:
                    drain(len(pending))
                else:
                    drain(-(-len(pending) // iters_left))

            # -- seq chunks c0..c3 --
            def kcs_for(ci):
                q0, W, nfull = CHUNKS[ci]
                full = [(kc, 0, None) for kc in range(nfull)]
                bound = []
                for j in range(5):
                    c0 = max(0, 128 * j - 8)
                    if c0 < W:
                        bound.append((nfull + j, c0, j))
                return full + bound

            def run_chunk(ci, drain_rate):
                q0, W, nfull = CHUNKS[ci]
                kcs = kcs_for(ci)
                last = len(kcs) - 1
                iters = 4 * len(kcs)
                for p in range(4):
                    avs = [
                        psAV.tile([128, 512], f32, tag="av", name=f"av{hl}")
                        for hl in range(2)
                    ]
                    sts = {}

                    def emit_S(idx):
                        kc, c0, _j = kcs[idx]
                        kw = min(128, L - 128 * kc)
                        st = psS.tile([128, 2, 512], f32, tag="st", name="stt")
                        for hl in range(2):
                            row = 64 * hl
                            nc.tensor.matmul(
                                st[:kw, hl, c0:W],
                                qkT[4 + p][row : row + 64, 128 * kc : 128 * kc + kw],
                                qkT[p][row : row + 64, q0 + c0 : q0 + W],
                                start=True,
                                stop=True,
                            )
                        sts[idx] = st

                    def emit_exp_av(idx):
                        kc, c0, j = kcs[idx]
                        kw = min(128, L - 128 * kc)
                        se = se_pool.tile([128, 2, 512], bf16, tag="se", name="set")
                        nc.scalar.activation(
                            se[:kw, :, c0:W],
                            sts.pop(idx)[:kw, :, c0:W],
                            EXP,
                            scale=0.125,
                        )
                        if j is not None:
                            nc.vector.tensor_mul(
                                se[:kw, :, c0:W],
                                se[:kw, :, c0:W],
                                mk[j][:kw, :, c0:W],
                            )
                        for hl in range(2):
                            nc.tensor.matmul(
                                avs[hl][: HD + 1, c0:W],
                                vt[kc][:kw, 2 * p + hl, :],
                                se[:kw, hl, c0:W],
                                start=(idx == 0),
                                stop=(idx == last),
                            )

                    emit_S(0)
                    if last >= 1:
                        emit_S(1)
                    for idx in range(len(kcs)):
                        emit_exp_av(idx)
                        if idx + 2 <= last:
                            emit_S(idx + 2)
                        drain(drain_rate)
                    normalize(avs, p, q0, W)

            # -- c4: 136 cols = 8 memory queries + last 128 seq queries --
            # key-chunk groups of 3 share one PSUM tile + one batched exp.
            C4G = [(0, 3), (3, 3), (6, 3), (9, 3), (12, 3), (15, 2)]

            def run_c4(drain_rate):
                iters = 4 * len(C4G)
                for p in range(4):
                    avs = [
                        psAV.tile([128, 512], f32, tag="av", name=f"av{hl}")
                        for hl in range(2)
                    ]
                    gts = {}

                    # sub-slot s lives at columns [s*C4W, (s+1)*C4W) inside the
                    # standard [128, 2, 512] tile: slots stay inside one PSUM
                    # bank per head (3*136 = 408 <= 512) so no matmul output
                    # ever straddles a bank boundary.
                    def emit_S4(g):
                        g0, gn = C4G[g]
                        st = psS.tile([128, 2, 512], f32, tag="st", name="st4t")
                        for s in range(gn):
                            kc = g0 + s
                            kw = min(128, L - 128 * kc)
                            for hl in range(2):
                                row = 64 * hl
                                nc.tensor.matmul(
                                    st[:kw, hl, s * C4W : (s + 1) * C4W],
                                    qkT[4 + p][
                                        row : row + 64, 128 * kc : 128 * kc + kw
                                    ],
                                    qm[p][row : row + 64, :],
                                    start=True,
                                    stop=True,
                                )
                        gts[g] = st

                    def emit_exp_av4(g):
                        g0, gn = C4G[g]
                        st = gts.pop(g)
                        se = se_pool.tile([128, 2, 512], bf16, tag="se", name="se4t")
                        nc.scalar.activation(
                            se[:, :, 0 : gn * C4W],
                            st[:, :, 0 : gn * C4W],
                            EXP,
                            scale=0.125,
                        )
                        for s in range(gn):
                            kc = g0 + s
                            kw = min(128, L - 128 * kc)
                            if kc >= 15:  # boundary masks for kc 15, 16
                                nc.vector.tensor_mul(
                                    se[:kw, :, s * C4W : (s + 1) * C4W],
                                    se[:kw, :, s * C4W : (s + 1) * C4W],
                                    mk4[kc - 15][:kw, :, :],
                                )
                            for hl in range(2):
                                nc.tensor.matmul(
                                    avs[hl][: HD + 1, 0:C4W],
                                    vt[kc][:kw, 2 * p + hl, :],
                                    se[:kw, hl, s * C4W : (s + 1) * C4W],
                                    start=(kc == 0),
                                    stop=(kc == NKC - 1),
                                )

                    emit_S4(0)
                    emit_S4(1)
                    for g in range(len(C4G)):
                        emit_exp_av4(g)
                        if g + 2 < len(C4G):
                            emit_S4(g + 2)
                        drain(drain_rate)
                    normalize_c4(avs, p)

            # ---- schedule ----
            # c0 stretch: rest of phase 1 for c1
            for m in MORD:
                pending += fillers_1b(640, 512, m)
            for l in range(5, 9):
                pending += fillers_1a(l)
            run_chunk(0, 6)
            drain(len(pending))

            # c1 stretch: c0's c_proj + phase 1 for c2
            u0, copy_c0 = phase3_seg(*SEGS[0], key="c0")
            pending += u0
            for m in MORD:
                pending += fillers_1b(1152, 512, m)
            for l in range(9, 13):
                pending += fillers_1a(l)
            run_chunk(1, 5)
            drain(len(pending))

            # c2 stretch: c1's c_proj + rest of phase 1 (tokens to 2056)
            u1, copy_c1 = phase3_seg(*SEGS[1], key="c1")
            pending += u1
            for m in MORD:
                pending += fillers_1b(1664, 392, m)
            for l in range(13, NKC):
                pending += fillers_1a(l)
            run_chunk(2, 5)
            drain(len(pending))

            # c3 stretch: c2's c_proj + c4 query staging + c0's out copy
            u2, copy_c2 = phase3_seg(*SEGS[2], key="c2")
            pending += u2
            pending.append(copy_c0)

            def stage_qm(p):
                def run():
                    nc.vector.tensor_copy(qm[p][:, 0:KMEM], qkT[p][:, 0:KMEM])
                    nc.vector.tensor_copy(qm[p][:, KMEM:C4W], qkT[p][:, 1928:2056])

                return run

            pending += [stage_qm(p) for p in range(4)]
            run_chunk(3, 2)
            drain(len(pending))

            # c4 stretch: c3's c_proj + c1/c2 out copies
            u3, copy_c3 = phase3_seg(*SEGS[3], key="c3")
            pending += u3
            pending.append(copy_c1)
            pending.append(copy_c2)
            run_c4(5)
            drain(len(pending))

            # tail: c4's c_proj + final small RS + last out copies
            u4, copy_c4 = phase3_seg(*SEGS[4], key="c4")
            for u in u4:
                u()
            copy_c3()
            copy_c4()


def _build():
    if "nc" in _cache:
        return _cache["nc"]
    import concourse.tile as tile
    from concourse import bacc, mybir

    bf16 = mybir.dt.bfloat16
    nc = bacc.Bacc(
        "TRN2",
        target_bir_lowering=False,
        debug=False,
        enable_asserts=False,
        num_devices=8,
    )
    xT = nc.dram_tensor("xT", [C, L], bf16, kind="ExternalInput").ap()
    w_qk = nc.dram_tensor("w_qk", [C, 1024], bf16, kind="ExternalInput").ap()
    w_v = nc.dram_tensor("w_v", [C, 512], bf16, kind="ExternalInput").ap()
    w_p = nc.dram_tensor("w_p", [512, C], bf16, kind="ExternalInput").ap()
    masks_d = nc.dram_tensor("masks", [5, 128, 1024], bf16, kind="ExternalInput").ap()
    maskc4_d = nc.dram_tensor(
        "maskc4", [2, 128, 2 * C4W], bf16, kind="ExternalInput"
    ).ap()
    out_ext = nc.dram_tensor("out", [OUT_ROWS, C], bf16, kind="ExternalOutput").ap()
    with tile.TileContext(nc) as tc:
        _emit(tc, xT, w_qk, w_v, w_p, masks_d, maskc4_d, out_ext)
    nc.compile()
    _cache["nc"] = nc
    return nc


def _make_masks():
    import ml_dtypes

    r_idx = np.arange(128)[:, None]
    c_idx = np.arange(512)[None, :]
    m = np.stack([(r_idx <= c_idx + 8 - 128 * j) for j in range(5)])
    return np.concatenate([m, m], axis=-1).astype(ml_dtypes.bfloat16)


def _make_maskc4():
    import ml_dtypes

    r = np.arange(128)[:, None]
    c = np.arange(C4W)[None, :]
    mem = c < KMEM
    m0 = mem | (r <= c)  # kc15: keys 1920:2048 vs seq queries 1928+(c-8)
    m1 = mem | (r <= c - 128)  # kc16: keys 2048:2056
    m = np.stack([m0, m1])
    return np.concatenate([m, m], axis=-1).astype(ml_dtypes.bfloat16)


def _bf16np():
    import ml_dtypes

    return ml_dtypes.bfloat16


def kernel(x, W_attn, W_proj, n_head, n_memory, _run_kw=None):
    x = np.asarray(x, dtype=np.float32)
    W_attn = np.asarray(W_attn, dtype=np.float32)
    W_proj = np.asarray(W_proj, dtype=np.float32)
    assert int(n_head) == 16 and int(n_memory) == KMEM
    assert x.shape == (B, L, C)

    from concourse.bass_utils import run_bass_kernel_spmd

    nc = _build()
    bf = _bf16np()
    masks = _make_masks()
    maskc4 = _make_maskc4()
    in_maps = []
    for core in range(8):
        b, hg = core // 2, core % 2
        s = slice(hg * 512, (hg + 1) * 512)
        in_maps.append(
            {
                "xT": np.ascontiguousarray(x[b].T).astype(bf),
                "w_qk": np.ascontiguousarray(
                    np.concatenate([W_attn[:, s], W_attn[:, 1024:2048][:, s]], axis=1)
                ).astype(bf),
                "w_v": np.ascontiguousarray(W_attn[:, 2048:3072][:, s]).astype(bf),
                "w_p": np.ascontiguousarray(W_proj[s, :]).astype(bf),
                "masks": masks,
                "maskc4": maskc4,
            }
        )
    res = run_bass_kernel_spmd(nc, in_maps, core_ids=list(range(8)), **(_run_kw or {}))
    out = np.empty((B, L, C), dtype=np.float32)
    for b in range(B):
        lo = np.asarray(res.results[2 * b]["out"], dtype=np.float32)
        hi = np.asarray(res.results[2 * b + 1]["out"], dtype=np.float32)
        for pieces, R, off in SEGS:
            h = R // 2
            # rank0 half = first h rows of the concatenated pieces, rank1 = rest
            rows = []
            for row0, nrows in pieces:
                rows.extend(range(row0, row0 + nrows))
            halves = [(lo, rows[:h], off), (hi, rows[h:], off)]
            for src, rlist, base in halves:
                i = 0
                while i < len(rlist):
                    j = i
                    while j + 1 < len(rlist) and rlist[j + 1] == rlist[j] + 1:
                        j += 1
                    out[b, rlist[i] : rlist[j] + 1] = src[base + i : base + j + 1]
                    i = j + 1
    if _run_kw:
        kernel.last_results = res
    return out


# revision 37
# speedup vs baseline: 1.0247x; 1.0247x over previous
"""CausalSelfAttentionWithMemory on 8 TRN2 NeuronCores — v3.

Sharding: core = 2*b + hg  (b in 0..3 batches, hg in 0..1 head-groups of 8
heads).  Each core computes qkv for its batch/head-group, attention, and the
partial c_proj (its 512 rows of W_proj); partials are pair-reduced with
chunked bf16 ReduceScatters; rs -> out copies are scheduled two stretches
later so no DMA queue ever waits behind an in-flight collective.

v3 changes vs v2 (374us -> 344us):
  - chunk order c0..c4 by ascending key range: attention (and the scalar
    engine's exp stream) starts right after a minimal upfront projection
    (tokens 0:640 only, wq loaded in two m-group halves); the rest of
    phase 1 drains as PE filler inside the early attention stretches,
    c_proj of chunk i drains inside chunk i+1.
  - last 512 queries split 384 (c3) + 128 (c4) so c3's ReduceScatter hides
    under c4's attention; only c4's small RS (136 rows) is exposed as tail,
    whose c_proj cycles PSUM through the freed attention slots.
  - the 8 memory queries are folded into c4 as extra score columns (same
    key range: everything) — removes v2's 272 tiny memq matmuls.
  - c4's exp is batched 3 key-chunks per ACTIVATE; sub-slots are packed
    inside the standard [128,2,512] PSUM tile because a matmul output must
    never straddle a 2KB PSUM bank boundary (hardware fault otherwise).
  - reciprocal_approx_fast/partition_broadcast run at the proven 512 width
    (width 136 hard-faulted the device; lanes beyond C4W are never read).

Known non-fixables found in profiling: the PE is power-throttled to 50%
for ~16% of the run (throttle_activity_1), and ACTIVATE has a ~250ns fixed
cost per instruction; PE busy ~276us of the ~344us wall.

Layouts on device (per core):
  xT    (1024, 2056) bf16  x[b] transposed (contraction dim on partitions)
  w_qk  (1024, 1024) bf16  [q-cols | k-cols] of this head-group
  w_v   (1024, 512)  bf16  v-cols
  w_p   (512, 1024)  bf16  W_proj rows of this head-group
  masks (5, 128, 1024) bf16 boundary masks: mask[j][r, hl*512+c] = r <= c+8-128j
  maskc4 (2, 128, 272) bf16 c4 masks over cols [mem 0:8 | seq 1928:2056]

Attention in S^T (key, query) layout: S^T tile = matmul(lhsT=k^T, rhs=q^T);
exp without max-subtraction (scores bounded); multiplicative causal mask;
AV via lhsT=[v|1] augmented (ones column = softmax denominator, row 64).
"""

import numpy as np

B, L, C = 4, 2056, 1024
HD = 64
KMEM = 8
HPC = 8  # heads per core
NKC = 17  # key chunks of 128 (last has 8 rows)

_cache = {}

# chunks: (q0, W, nfull) for the four seq chunks; c4 is special.
CHUNKS = [(8, 512, 0), (520, 512, 4), (1032, 512, 8), (1544, 384, 12)]
# c_proj/RS segments: seg[c] = (pieces, R, out_off); pieces = [(row0, nrows)].
SEGS = [
    ([(8, 512)], 512, 0),
    ([(520, 512)], 512, 256),
    ([(1032, 512)], 512, 512),
    ([(1544, 384)], 384, 768),
    ([(0, 8), (1928, 128)], 136, 960),
]
OUT_ROWS = 1028
C4W = 136  # 8 mem + 128 seq query columns


def _emit(tc, xT, w_qk, w_v, w_p, masks_d, maskc4_d, out_ext):
    import concourse.bass as bass  # noqa: F401
    from concourse import mybir

    nc = tc.nc
    f32 = mybir.dt.float32
    bf16 = mybir.dt.bfloat16
    EXP = mybir.ActivationFunctionType.Exp

    with (
        tc.tile_pool(name="res", bufs=1) as res_pool,
        tc.tile_pool(name="dram", bufs=1, space="DRAM") as d_pool,
    ):
        # ---- resident tensors; DMAs ordered to unblock the upfront phase ----
        wv = [res_pool.tile([128, 512], bf16, name=f"wv{cc}") for cc in range(8)]
        for cc in range(8):
            nc.sync.dma_start(wv[cc], w_v[128 * cc : 128 * cc + 128, :])
        x_sb = res_pool.tile([128, 8, L], bf16, name="x_sb")
        for cc in range(8):  # tokens 0:512 first so 1a can start early
            nc.sync.dma_start(x_sb[:, cc, 0:512], xT[128 * cc : 128 * cc + 128, 0:512])
        # wq split in two half-loads: pairs 0/1 columns (m 0,4,1,5) first so
        # the first 1b matmuls start after 1MB instead of 2MB.
        wq = [res_pool.tile([128, 1024], bf16, name=f"wq{cc}") for cc in range(8)]
        for cc in range(8):
            nc.sync.dma_start(
                wq[cc].rearrange("p (h c) -> p h c", h=2)[:, :, 0:256],
                w_qk[128 * cc : 128 * cc + 128, :]
                .rearrange("p (h c) -> p h c", h=2)[:, :, 0:256],
            )
        mk = [res_pool.tile([128, 2, 512], bf16, name=f"mk{j}") for j in range(5)]
        for cc in range(8):
            nc.sync.dma_start(
                x_sb[:, cc, 512:640], xT[128 * cc : 128 * cc + 128, 512:640]
            )
        for cc in range(8):
            nc.sync.dma_start(
                wq[cc].rearrange("p (h c) -> p h c", h=2)[:, :, 256:512],
                w_qk[128 * cc : 128 * cc + 128, :]
                .rearrange("p (h c) -> p h c", h=2)[:, :, 256:512],
            )
        for j in range(5):
            nc.sync.dma_start(mk[j].rearrange("p a b -> p (a b)"), masks_d[j])
        mk4 = [res_pool.tile([128, 2, C4W], bf16, name=f"mk4{j}") for j in range(2)]
        for j in range(2):
            nc.sync.dma_start(mk4[j].rearrange("p a b -> p (a b)"), maskc4_d[j])
        wp = [res_pool.tile([128, 1024], bf16, name=f"wp{rr}") for rr in range(4)]
        for rr in range(4):
            nc.sync.dma_start(wp[rr], w_p[128 * rr : 128 * rr + 128, :])
        for n0, nw in [(640, 512), (1152, 512), (1664, 384), (2048, 8)]:
            for cc in range(8):
                nc.sync.dma_start(
                    x_sb[:, cc, n0 : n0 + nw],
                    xT[128 * cc : 128 * cc + 128, n0 : n0 + nw],
                )

        qkT = [res_pool.tile([128, L], bf16, name=f"qkT{m}") for m in range(8)]
        vt = [res_pool.tile([128, HPC, HD + 1], bf16, name=f"vt{l}") for l in range(NKC)]
        yt = [res_pool.tile([128, L], bf16, name=f"yt{p}") for p in range(4)]
        qm = [res_pool.tile([128, C4W], bf16, name=f"qm{p}") for p in range(4)]

        MORD = [0, 4, 1, 5, 2, 6, 3, 7]  # q/k tiles of pair p land together

        # ---- upfront: only what chunk c0 needs (q/k tokens 0:640, v 0:640) ----
        with tc.tile_pool(name="ps1", bufs=4, space="PSUM") as ps1:
            # PE warmup during the initial DMA wait lifts HAM to K=8/8.
            warm = res_pool.tile([128, 512], bf16, name="warm")
            nc.vector.memset(warm, 0.0)
            pw = ps1.tile([128, 512], f32, tag="ps", name="pw")
            for i in range(16):
                nc.tensor.matmul(
                    pw, warm[:, 0:128], warm, start=(i == 0), stop=(i == 15)
                )

            def p1a(l):
                lw = min(128, L - 128 * l)
                ps = ps1.tile([128, 512], f32, tag="ps", name="pst")
                for cc in range(8):
                    nc.tensor.matmul(
                        ps[:lw, :],
                        x_sb[:, cc, 128 * l : 128 * l + lw],
                        wv[cc],
                        start=(cc == 0),
                        stop=(cc == 7),
                    )
                nc.vector.tensor_copy(
                    vt[l][:lw, :, 0:HD],
                    ps[:lw, :].rearrange("p (h d) -> p h d", h=HPC),
                )
                nc.vector.memset(vt[l][:, :, HD : HD + 1], 1.0)

            def p1b(n0, nw, m):
                ps = ps1.tile([128, 512], f32, tag="ps", name="pst")
                for cc in range(8):
                    nc.tensor.matmul(
                        ps[:, :nw],
                        wq[cc][:, 128 * m : 128 * m + 128],
                        x_sb[:, cc, n0 : n0 + nw],
                        start=(cc == 0),
                        stop=(cc == 7),
                    )
                nc.vector.tensor_copy(qkT[m][:, n0 : n0 + nw], ps[:, :nw])

            for l in range(5):
                p1a(l)
            for m in MORD:
                p1b(0, 512, m)
            for m in MORD:
                p1b(512, 128, m)

        # ---- attention chunks + chunked c_proj/ReduceScatter ----
        with (
            tc.tile_pool(name="psS", bufs=2, space="PSUM") as psS,
            tc.tile_pool(name="psAV", bufs=3, space="PSUM") as psAV,
            tc.tile_pool(name="ps3", bufs=1, space="PSUM") as ps3,
            tc.tile_pool(name="sexp", bufs=4) as se_pool,
            tc.tile_pool(name="small", bufs=2) as sm_pool,
            tc.tile_pool(name="ostage", bufs=4) as o_pool,
        ):
            # -- phase-1 remainder as single-instruction filler closures --
            def fillers_1a(l):
                lw = min(128, L - 128 * l)
                ps = ps3.tile([128, 512], f32, tag="p3", name="pft")

                def mm(cc):
                    def run():
                        nc.tensor.matmul(
                            ps[:lw, :],
                            x_sb[:, cc, 128 * l : 128 * l + lw],
                            wv[cc],
                            start=(cc == 0),
                            stop=(cc == 7),
                        )

                    return run

                def fin():
                    nc.vector.tensor_copy(
                        vt[l][:lw, :, 0:HD],
                        ps[:lw, :].rearrange("p (h d) -> p h d", h=HPC),
                    )
                    nc.vector.memset(vt[l][:, :, HD : HD + 1], 1.0)

                return [mm(cc) for cc in range(8)] + [fin]

            def fillers_1b(n0, nw, m):
                ps = ps3.tile([128, 512], f32, tag="p3", name="pft")

                def mm(cc):
                    def run():
                        nc.tensor.matmul(
                            ps[:, :nw],
                            wq[cc][:, 128 * m : 128 * m + 128],
                            x_sb[:, cc, n0 : n0 + nw],
                            start=(cc == 0),
                            stop=(cc == 7),
                        )

                    return run

                def fin():
                    nc.vector.tensor_copy(qkT[m][:, n0 : n0 + nw], ps[:, :nw])

                return [mm(cc) for cc in range(8)] + [fin]

            def normalize(avs, p, q0, qw):
                """yt[p][:, q0:q0+qw] = avs rows/den (row 64 = denominator)."""
                for hl in range(2):
                    den = sm_pool.tile([1, 512], f32, tag="den", name="dent")
                    nc.vector.tensor_copy(den[:, :qw], avs[hl][HD : HD + 1, :qw])
                    inv = sm_pool.tile([1, 512], f32, tag="inv", name="invt")
                    nc.vector.reciprocal_approx_fast(inv[:, :qw], den[:, :qw])
                    bc = sm_pool.tile([64, 512], f32, tag="bc", name="bct")
                    nc.gpsimd.partition_broadcast(bc[:, :qw], inv[:, :qw])
                    nc.vector.tensor_mul(
                        yt[p][64 * hl : 64 * hl + 64, q0 : q0 + qw],
                        avs[hl][0:HD, :qw],
                        bc[:, :qw],
                    )

            def normalize_c4(avs, p):
                """c4 columns map to yt cols [0:8 | 1928:2056].  den/recip/
                broadcast run at the v2-proven 512 width (lanes beyond C4W
                hold garbage and are never read)."""
                for hl in range(2):
                    den = sm_pool.tile([1, 512], f32, tag="den", name="dent")
                    nc.vector.tensor_copy(den, avs[hl][HD : HD + 1, :])
                    inv = sm_pool.tile([1, 512], f32, tag="inv", name="invt")
                    nc.vector.reciprocal_approx_fast(inv, den)
                    bc = sm_pool.tile([64, 512], f32, tag="bc", name="bct")
                    nc.gpsimd.partition_broadcast(bc, inv)
                    row = 64 * hl
                    nc.vector.tensor_mul(
                        yt[p][row : row + 64, 0:KMEM],
                        avs[hl][0:HD, 0:KMEM],
                        bc[:, 0:KMEM],
                    )
                    nc.vector.tensor_mul(
                        yt[p][row : row + 64, 1928:2056],
                        avs[hl][0:HD, KMEM:C4W],
                        bc[:, KMEM:C4W],
                    )

            def phase3_seg(pieces, R, out_off, key):
                """c_proj for the given row pieces as single-matmul closures,
                then a ReduceScatter into an internal DRAM tile.  The rs ->
                out_ext copy is scheduled later (copies list) so no DMA queue
                ever waits behind an in-flight collective."""
                partial = d_pool.tile([R, C], bf16, name=f"partial{key}")
                units = []
                tlist = []  # (src_row0, dst_row0, lw)
                dst = 0
                for row0, nrows in pieces:
                    for t0 in range(0, nrows, 128):
                        lw = min(128, nrows - t0)
                        tlist.append((row0 + t0, dst, lw))
                        dst += lw
                psum_cycle = [(ps3, "p3")]
                if key == "c4":  # attention pools are free during the tail
                    psum_cycle = [(ps3, "p3"), (psS, "st"), (psS, "st")]
                ci = 0
                for src0, dst0, lw in tlist:
                    for n in range(2):
                        pool, ptag = psum_cycle[ci % len(psum_cycle)]
                        ci += 1
                        ps = pool.tile([128, 512], f32, tag=ptag, name="ps3t")

                        def mm(rr, ps=ps, src0=src0, n=n, lw=lw):
                            def run():
                                nc.tensor.matmul(
                                    ps[:lw, :],
                                    yt[rr][:, src0 : src0 + lw],
                                    wp[rr][:, 512 * n : 512 * n + 512],
                                    start=(rr == 0),
                                    stop=(rr == 3),
                                )

                            return run

                        def fin(ps=ps, dst0=dst0, n=n, lw=lw):
                            ost = o_pool.tile([128, 512], bf16, tag="ost", name="ostt")
                            nc.vector.tensor_copy(ost[:lw, :], ps[:lw, :])
                            nc.sync.dma_start(
                                partial[dst0 : dst0 + lw, 512 * n : 512 * n + 512],
                                ost[:lw, :],
                            )

                        units += [mm(rr) for rr in range(4)]
                        units.append(fin)

                rs = d_pool.tile([R // 2, C], bf16, name=f"rs{key}")

                def finish():
                    nc.gpsimd.collective_compute(
                        "ReduceScatter",
                        mybir.AluOpType.add,
                        replica_groups=[[0, 1], [2, 3], [4, 5], [6, 7]],
                        ins=[partial.opt()],
                        outs=[rs.opt()],
                    )

                def copy_out():
                    nc.sync.dma_start(out_ext[out_off : out_off + R // 2, :], rs)

                units.append(finish)
                return units, copy_out

            pending = []  # filler closures drained into the attention stream

            def drain(n):
                for _ in range(min(n, len(pending))):
                    pending.pop(0)()

            def drain_paced(iters_left):
                """Spread the remaining fillers evenly over the remaining
                exp/AV iterations of the stretch so the PE never runs dry."""
                if iters_left <= 1:
                    drain(len(pending))
                else:
                    drain(-(-len(pending) // iters_left))

            # -- seq chunks c0..c3 --
            def kcs_for(ci):
                q0, W, nfull = CHUNKS[ci]
                full = [(kc, 0, None) for kc in range(nfull)]
                bound = []
                for j in range(5):
                    c0 = max(0, 128 * j - 8)
                    if c0 < W:
                        bound.append((nfull + j, c0, j))
                return full + bound

            def run_chunk(ci, drain_rate):
                q0, W, nfull = CHUNKS[ci]
                kcs = kcs_for(ci)
                last = len(kcs) - 1
                iters = 4 * len(kcs)
                for p in range(4):
                    avs = [
                        psAV.tile([128, 512], f32, tag="av", name=f"av{hl}")
                        for hl in range(2)
                    ]
                    sts = {}

                    def emit_S(idx):
                        kc, c0, _j = kcs[idx]
                        kw = min(128, L - 128 * kc)
                        st = psS.tile([128, 2, 512], f32, tag="st", name="stt")
                        for hl in range(2):
                            row = 64 * hl
                            nc.tensor.matmul(
                                st[:kw, hl, c0:W],
                                qkT[4 + p][row : row + 64, 128 * kc : 128 * kc + kw],
                                qkT[p][row : row + 64, q0 + c0 : q0 + W],
                                start=True,
                                stop=True,
                            )
                        sts[idx] = st

                    def emit_exp_av(idx):
                        kc, c0, j = kcs[idx]
                        kw = min(128, L - 128 * kc)
                        se = se_pool.tile([128, 2, 512], bf16, tag="se", name="set")
                        nc.scalar.activation(
                            se[:kw, :, c0:W],
                            sts.pop(idx)[:kw, :, c0:W],
                            EXP,
                            scale=0.125,
                        )
                        if j is not None:
                            nc.vector.tensor_mul(
                                se[:kw, :, c0:W],
                                se[:kw, :, c0:W],
                                mk[j][:kw, :, c0:W],
                            )
                        for hl in range(2):
                            nc.tensor.matmul(
                                avs[hl][: HD + 1, c0:W],
                                vt[kc][:kw, 2 * p + hl, :],
                                se[:kw, hl, c0:W],
                                start=(idx == 0),
                                stop=(idx == last),
                            )

                    emit_S(0)
                    if last >= 1:
                        emit_S(1)
                    for idx in range(len(kcs)):
                        emit_exp_av(idx)
                        if idx + 2 <= last:
                            emit_S(idx + 2)
                        drain(drain_rate)
                    normalize(avs, p, q0, W)

            # -- c4: 136 cols = 8 memory queries + last 128 seq queries --
            # key-chunk groups of 3 share one PSUM tile + one batched exp.
            C4G = [(0, 3), (3, 3), (6, 3), (9, 3), (12, 3), (15, 2)]

            def run_c4(drain_rate):
                iters = 4 * len(C4G)
                for p in range(4):
                    avs = [
                        psAV.tile([128, 512], f32, tag="av", name=f"av{hl}")
                        for hl in range(2)
                    ]
                    gts = {}

                    # sub-slot s lives at columns [s*C4W, (s+1)*C4W) inside the
                    # standard [128, 2, 512] tile: slots stay inside one PSUM
                    # bank per head (3*136 = 408 <= 512) so no matmul output
                    # ever straddles a bank boundary.
                    def emit_S4(g):
                        g0, gn = C4G[g]
                        st = psS.tile([128, 2, 512], f32, tag="st", name="st4t")
                        for s in range(gn):
                            kc = g0 + s
                            kw = min(128, L - 128 * kc)
                            for hl in range(2):
                                row = 64 * hl
                                nc.tensor.matmul(
                                    st[:kw, hl, s * C4W : (s + 1) * C4W],
                                    qkT[4 + p][
                                        row : row + 64, 128 * kc : 128 * kc + kw
                                    ],
                                    qm[p][row : row + 64, :],
                                    start=True,
                                    stop=True,
                                )
                        gts[g] = st

                    def emit_exp_av4(g):
                        g0, gn = C4G[g]
                        st = gts.pop(g)
                        se = se_pool.tile([128, 2, 512], bf16, tag="se", name="se4t")
                        nc.scalar.activation(
                            se[:, :, 0 : gn * C4W],
                            st[:, :, 0 : gn * C4W],
                            EXP,
                            scale=0.125,
                        )
                        for s in range(gn):
                            kc = g0 + s
                            kw = min(128, L - 128 * kc)
                            if kc >= 15:  # boundary masks for kc 15, 16
                                nc.vector.tensor_mul(
                                    se[:kw, :, s * C4W : (s + 1) * C4W],
                                    se[:kw, :, s * C4W : (s + 1) * C4W],
                                    mk4[kc - 15][:kw, :, :],
                                )
                            for hl in range(2):
                                nc.tensor.matmul(
                                    avs[hl][: HD + 1, 0:C4W],
                                    vt[kc][:kw, 2 * p + hl, :],
                                    se[:kw, hl, s * C4W : (s + 1) * C4W],
                                    start=(kc == 0),
                                    stop=(kc == NKC - 1),
                                )

                    emit_S4(0)
                    emit_S4(1)
                    for g in range(len(C4G)):
                        emit_exp_av4(g)
                        if g + 2 < len(C4G):
                            emit_S4(g + 2)
                        drain(drain_rate)
                    normalize_c4(avs, p)

            # ---- schedule ----
            # c0 stretch: rest of phase 1 for c1
            for m in MORD:
                pending += fillers_1b(640, 512, m)
            for l in range(5, 9):
                pending += fillers_1a(l)
            run_chunk(0, 6)
            drain(len(pending))

            # c1 stretch: c0's c_proj + phase 1 for c2
            u0, copy_c0 = phase3_seg(*SEGS[0], key="c0")
            pending += u0
            for m in MORD:
                pending += fillers_1b(1152, 512, m)
            for l in range(9, 13):
                pending += fillers_1a(l)
            run_chunk(1, 5)
            drain(len(pending))

            # c2 stretch: c1's c_proj + rest of phase 1 (tokens to 2056)
            u1, copy_c1 = phase3_seg(*SEGS[1], key="c1")
            pending += u1
            for m in MORD:
                pending += fillers_1b(1664, 392, m)
            for l in range(13, NKC):
                pending += fillers_1a(l)
            run_chunk(2, 5)
            drain(len(pending))

            # c3 stretch: c2's c_proj + c4 query staging + c0's out copy
            u2, copy_c2 = phase3_seg(*SEGS[2], key="c2")
            pending += u2
            pending.append(copy_c0)

            def stage_qm(p):
                def run():
                    nc.vector.tensor_copy(qm[p][:, 0:KMEM], qkT[p][:, 0:KMEM])
                    nc.vector.tensor_copy(qm[p][:, KMEM:C4W], qkT[p][:, 1928:2056])

                return run

            pending += [stage_qm(p) for p in range(4)]
            run_chunk(3, 2)
            drain(len(pending))

            # c4 stretch: c3's c_proj + c1/c2 out copies
            u3, copy_c3 = phase3_seg(*SEGS[3], key="c3")
            pending += u3
            pending.append(copy_c1)
            pending.append(copy_c2)
            run_c4(5)
            drain(len(pending))

            # tail: c4's c_proj + final small RS + last out copies
            u4, copy_c4 = phase3_seg(*SEGS[4], key="c4")
            for u in u4:
                u()
            copy_c3()
            copy_c4()


def _build():
    if "nc" in _cache:
        return _cache["nc"]
    import concourse.tile as tile
    from concourse import bacc, mybir

    bf16 = mybir.dt.bfloat16
    nc = bacc.Bacc(
        "TRN2",
        target_bir_lowering=False,
        debug=False,
        enable_asserts=False,
        num_devices=8,
    )
    xT = nc.dram_tensor("xT", [C, L], bf16, kind="ExternalInput").ap()
    w_qk = nc.dram_tensor("w_qk", [C, 1024], bf16, kind="ExternalInput").ap()
    w_v = nc.dram_tensor("w_v", [C, 512], bf16, kind="ExternalInput").ap()
    w_p = nc.dram_tensor("w_p", [512, C], bf16, kind="ExternalInput").ap()
    masks_d = nc.dram_tensor("masks", [5, 128, 1024], bf16, kind="ExternalInput").ap()
    maskc4_d = nc.dram_tensor(
        "maskc4", [2, 128, 2 * C4W], bf16, kind="ExternalInput"
    ).ap()
    out_ext = nc.dram_tensor("out", [OUT_ROWS, C], bf16, kind="ExternalOutput").ap()
    with tile.TileContext(nc) as tc:
        _emit(tc, xT, w_qk, w_v, w_p, masks_d, maskc4_d, out_ext)
    nc.compile()
    _cache["nc"] = nc
    return nc


def _make_masks():
    import ml_dtypes

    r_idx = np.arange(128)[:, None]
    c_idx = np.arange(512)[None, :]
    m = np.stack([(r_idx <= c_idx + 8 - 128 * j) for j in range(5)])
    return np.concatenate([m, m], axis=-1).astype(ml_dtypes.bfloat16)


def _make_maskc4():
    import ml_dtypes

    r = np.arange(128)[:, None]
    c = np.arange(C4W)[None, :]
    mem = c < KMEM
    m0 = mem | (r <= c)  # kc15: keys 1920:2048 vs seq queries 1928+(c-8)
    m1 = mem | (r <= c - 128)  # kc16: keys 2048:2056
    m = np.stack([m0, m1])
    return np.concatenate([m, m], axis=-1).astype(ml_dtypes.bfloat16)


def _bf16np():
    import ml_dtypes

    return ml_dtypes.bfloat16


def kernel(x, W_attn, W_proj, n_head, n_memory, _run_kw=None):
    x = np.asarray(x, dtype=np.float32)
    W_attn = np.asarray(W_attn, dtype=np.float32)
    W_proj = np.asarray(W_proj, dtype=np.float32)
    assert int(n_head) == 16 and int(n_memory) == KMEM
    assert x.shape == (B, L, C)

    from concourse.bass_utils import run_bass_kernel_spmd

    nc = _build()
    bf = _bf16np()
    masks = _make_masks()
    maskc4 = _make_maskc4()
    in_maps = []
    for core in range(8):
        b, hg = core // 2, core % 2
        s = slice(hg * 512, (hg + 1) * 512)
        in_maps.append(
            {
                "xT": np.ascontiguousarray(x[b].T).astype(bf),
                "w_qk": np.ascontiguousarray(
                    np.concatenate([W_attn[:, s], W_attn[:, 1024:2048][:, s]], axis=1)
                ).astype(bf),
                "w_v": np.ascontiguousarray(W_attn[:, 2048:3072][:, s]).astype(bf),
                "w_p": np.ascontiguousarray(W_proj[s, :]).astype(bf),
                "masks": masks,
                "maskc4": maskc4,
            }
        )
    res = run_bass_kernel_spmd(nc, in_maps, core_ids=list(range(8)), **(_run_kw or {}))
    out = np.empty((B, L, C), dtype=np.float32)
    for b in range(B):
        lo = np.asarray(res.results[2 * b]["out"], dtype=np.float32)
        hi = np.asarray(res.results[2 * b + 1]["out"], dtype=np.float32)
        for pieces, R, off in SEGS:
            h = R // 2
            # rank0 half = first h rows of the concatenated pieces, rank1 = rest
            rows = []
            for row0, nrows in pieces:
                rows.extend(range(row0, row0 + nrows))
            halves = [(lo, rows[:h], off), (hi, rows[h:], off)]
            for src, rlist, base in halves:
                i = 0
                while i < len(rlist):
                    j = i
                    while j + 1 < len(rlist) and rlist[j + 1] == rlist[j] + 1:
                        j += 1
                    out[b, rlist[i] : rlist[j] + 1] = src[base + i : base + j + 1]
                    i = j + 1
    if _run_kw:
        kernel.last_results = res
    return out
